# revision 40
# baseline (speedup 1.0000x reference)
"""CirculantLinear kernel for Trainium2 (Bass/Tile), 8-core data-parallel.

Math: reference computes out = IFFT_ortho( FFT(eigens) * FFT_ortho(x) ) summed
over input blocks.  Because the ortho norms cancel against the unnormalized
eigen-FFT, this is exactly a block-circulant real matmul applied to both the
real and imaginary channel of x:

    out[b, y*8+i2, c] = sum_{gx,m} W[y*8+i2, gx*8+m] * x[b, gx*8+m, c]
    W[o, i] = eigens[o//8, i//8, (o-i) % 8]

so the whole op is a dense 1024x1024 matmul on the TensorEngine (fp16 operands,
fp32 PSUM accumulation), sharded over the batch dim across 8 NeuronCores.

The circulant expansion of eigens into W happens ON DEVICE: the host only
passes a doubled+transposed copy of eigens (layout transform), and 8 strided
VectorE copies build the fp16 weight tiles (the mod-8 wrap becomes a plain
sliding window on the doubled axis).
"""

import os
import sys

import numpy as np

# Runtime deps (available via PYTHONPATH in the container; add fallbacks so the
# kernel is importable from a bare directory too).
for _p in ("/root/.axon_site/_ro/trn_rl_repo", "/root/.axon_site/_ro/pypackages",
           "/opt/trn_rl_repo", "/opt/pypackages"):
    if os.path.isdir(_p) and _p not in sys.path:
        sys.path.append(_p)

import concourse.bacc as bacc
import concourse.mybir as mybir
import concourse.tile as tile
from concourse import bass_utils

# Problem constants (hardcoded per contract).
B, IN, OUT, K = 8192, 1024, 1024, 8
GX, GY = IN // K, OUT // K  # 128, 128
N_CORES = 8
B_CORE = B // N_CORES  # 1024
P = 128  # partitions

F32 = mybir.dt.float32
F16 = mybir.dt.float16


def build_nc(b_core: int = B_CORE):
    """Build the per-core Bass program (SPMD: same program, different x shard)."""
    assert b_core % P == 0
    n_bt = b_core // P

    nc = bacc.Bacc("TRN2", target_bir_lowering=False, debug=False)

    x_dram = nc.dram_tensor("x", [b_core, 2 * IN], F32, kind="ExternalInput")
    # e2t[gx, y*16 + d] = eigens[y, gx, d % 8]   (doubled k-axis, gx-major)
    e2t_dram = nc.dram_tensor("e2t", [GX, GY * 2 * K], F16, kind="ExternalInput")
    out_dram = nc.dram_tensor("out", [b_core, 2 * OUT], F32, kind="ExternalOutput")

    with tile.TileContext(nc) as tc:
        with (
            tc.tile_pool(name="const", bufs=1) as constp,
            tc.tile_pool(name="xraw", bufs=2) as xrawp,
            tc.tile_pool(name="xm", bufs=2) as xmp,
            tc.tile_pool(name="xt", bufs=2) as xtp,
            tc.tile_pool(name="outp", bufs=8) as outp,
            tc.tile_pool(name="psum", bufs=8, space="PSUM") as psump,
        ):
            # ALL DMAs ride the single SP (HWDGE) queue in an explicit static
            # order; the sequencer's head-of-line blocking turns that order
            # into a static schedule: e16, x0, T0, x1, T1, x2, x3, T2, ...
            # with every store deferred to the end (stores have no deadline).
            groups = [1, 2, 2, 2, 1] if n_bt == 8 else [1] * n_bt
            g_base = []
            b0 = 0
            for g in groups:
                g_base.append(b0)
                b0 += g

            e2t = constp.tile([P, GY * 2 * K], F16, name="e2t_sb")
            e2t_v = e2t.rearrange("g (y d) -> g y d", d=2 * K)  # [128,128,16]


            # first group's load up front (before e2t: x0 feeds the critical
            # cast->transpose chain); later loads are emitted after the
            # previous group's transpose (FIFO => auto-throttled prefetch)
            xraws = [None] * n_bt

            def load(bt):
                xr = xrawp.tile([P, 2 * IN], F32, name="xraw")
                nc.sync.dma_start(xr, x_dram.ap()[bt * P : (bt + 1) * P, :])
                xraws[bt] = xr

            for bt in range(g_base[1] if len(groups) > 1 else n_bt):
                load(bt)
            nc.scalar.dma_start(e2t, e2t_dram.ap())

            # PE warm-up: the DMA prologue leaves the PE idle for >10us which
            # would leave the HAM clock-gate cold for the first real matmul
            # group; burn dummy matmuls on a zeroed scratch meanwhile.
            n_warm = 16
            if n_warm:
                wscr = constp.tile([P, 640], F16, name="wscr")
                nc.scalar.memzero(wscr[:])
                wps = psump.tile([P, 512], F32, name="wps", tag="ps")
                for _ in range(n_warm):
                    nc.tensor.matmul(wps, wscr[:, 0:P],
                                     wscr[:, P : P + 512], start=True, stop=True)

            # weights wt_m[gx, y*8+i2] = e[y, gx, (i2-m)%8]: one tile per m so
            # each matmul group only depends on the chunks it reads; (i2-m)%8
            # becomes a plain sliding window on the doubled axis.  All eight
            # copies run on the DVE, emitted after the first group's casts.
            wt_mo = []
            for m in range(K):
                wtm = constp.tile([P, OUT], F16, name="wtm", tag=f"wtm{m}")
                wt_mo.append(wtm)

            def emit_wt(ms):
                for m in ms:
                    src = e2t_v[:, :, K - m : 2 * K - m]
                    dst = wt_mo[m].rearrange("g (y i) -> g y i", i=K)
                    nc.vector.tensor_copy(dst, src)

            stores = []  # deferred (out_dram_slice, out_sb_slice, engine)

            for gidx, g_sz in enumerate(groups):
                base = g_base[gidx]
                # casts: deinterleave re/im + fp32->fp16 + reorder
                #   in  free idx: gx*16 + m*2 + c
                #   out free idx: ((c*8+m)*G + gi)*128 + gx
                nbufs = min(2, groups.count(g_sz))
                xm = xmp.tile([P, g_sz * 2 * IN], F16, name="xm", tag=f"xm{g_sz}",
                              bufs=nbufs)
                xm_v = xm.rearrange("b (c m i g) -> b c m i g", c=2, m=K, i=g_sz)
                for gi in range(g_sz):
                    xraw_v = xraws[base + gi].rearrange(
                        "b (g m c) -> b c m g", g=GX, m=K, c=2
                    )
                    for c in range(2):
                        nc.vector.tensor_copy(xm_v[:, c, :, gi], xraw_v[:, c])
                if gidx == 0:
                    emit_wt(range(K))

                # xbar transpose(s): [b, f] -> [gx, f//128, b]; first group per
                # channel so c0's matmuls start asap
                xt = xtp.tile([P, g_sz * 2 * K, P], F16, name="xt", tag=f"xt{g_sz}",
                              bufs=nbufs)
                with tc.high_priority():
                    if gidx == 0:
                        half = g_sz * K * P
                        nc.sync.dma_start(xt[:, 0 : g_sz * K, :], xm[:, 0:half],
                                          transpose=True)
                        nc.sync.dma_start(xt[:, g_sz * K : 2 * g_sz * K, :],
                                          xm[:, half : 2 * half], transpose=True)
                    else:
                        nc.sync.dma_start(xt, xm, transpose=True)
                xt_v = xt.rearrange("g (c m i) b -> g c m i b", c=2, m=K)

                if gidx + 1 < len(groups):
                    nxt = g_base[gidx + 1]
                    for bt in range(nxt, nxt + groups[gidx + 1]):
                        load(bt)


                for gi in range(g_sz):
                    bt = base + gi
                    out_sb = outp.tile([P, 2 * OUT], F32, name="out_sb")
                    out_v = out_sb.rearrange("b (o c) -> b c o", c=2)
                    last = bt == n_bt - 1
                    for c in range(2):
                        ps0 = psump.tile([P, 512], F32, name="ps0", tag="ps")
                        ps1 = psump.tile([P, 512], F32, name="ps1", tag="ps")
                        if last and c == 1:
                            # de-interleave the final groups: ps0's evac+store
                            # overlaps ps1's matmuls (shorter kernel tail)
                            for m in range(K):
                                nc.tensor.matmul(ps0, xt_v[:, c, m, gi],
                                                 wt_mo[m][:, 0:512],
                                                 start=(m == 0), stop=(m == K - 1))
                            nc.scalar.copy(out_v[:, c, 0:512], ps0)
                            for m in range(K):
                                nc.tensor.matmul(ps1, xt_v[:, c, m, gi],
                                                 wt_mo[m][:, 512:1024],
                                                 start=(m == 0), stop=(m == K - 1))
                            nc.vector.tensor_copy(out_v[:, c, 512:1024], ps1)
                            continue
                        for m in range(K):
                            lhsT = xt_v[:, c, m, gi]  # [gx=128, b=128]
                            nc.tensor.matmul(
                                ps0, lhsT, wt_mo[m][:, 0:512],
                                start=(m == 0), stop=(m == K - 1),
                            )
                            nc.tensor.matmul(
                                ps1, lhsT, wt_mo[m][:, 512:1024],
                                start=(m == 0), stop=(m == K - 1),
                            )
                        # PSUM -> SBUF re-interleaving channels (strided write)
                        nc.scalar.copy(out_v[:, c, 0:512], ps0)
                        nc.scalar.copy(out_v[:, c, 512:1024], ps1)

                    n_pieces = 4 if last else 2
                    step = (2 * OUT) // n_pieces
                    for pc in range(n_pieces):
                        stores.append((
                            (bt * P, pc * step, (pc + 1) * step),
                            out_sb[:, pc * step : (pc + 1) * step],
                        ))

            # deferred stores, in order, at the back of the SP queue
            for (row0, c0_, c1_), src in stores:
                nc.sync.dma_start(out_dram.ap()[row0 : row0 + P, c0_:c1_], src)

    nc.compile()
    return nc


_NC_CACHE = {}


def _get_nc(b_core: int = B_CORE):
    if b_core not in _NC_CACHE:
        _NC_CACHE[b_core] = build_nc(b_core)
    return _NC_CACHE[b_core]


def make_e2t(eigens: np.ndarray) -> np.ndarray:
    """Host layout transform: doubled k-axis, gx-major.  No arithmetic."""
    e2 = np.concatenate([eigens, eigens], axis=-1)  # [GY, GX, 16]
    return np.ascontiguousarray(e2.transpose(1, 0, 2)).reshape(GX, GY * 2 * K).astype(np.float16)


def kernel(x: np.ndarray, eigens: np.ndarray, _trace: bool = False):
    assert x.shape == (B, IN, 2) and eigens.shape == (GY, GX, K)
    nc = _get_nc()
    e2t = make_e2t(np.asarray(eigens, dtype=np.float32))
    x = np.asarray(x, dtype=np.float32)
    in_maps = []
    for ci in range(N_CORES):
        xs = np.ascontiguousarray(x[ci * B_CORE : (ci + 1) * B_CORE]).reshape(B_CORE, 2 * IN)
        in_maps.append({"x": xs, "e2t": e2t})
    res = bass_utils.run_bass_kernel_spmd(
        nc, in_maps, core_ids=list(range(N_CORES)), trace=_trace
    )
    out = np.concatenate(
        [r["out"].reshape(B_CORE, OUT, 2) for r in res.results], axis=0
    )
    if _trace:
        return out, res
    return out



# revision 44
# speedup vs baseline: 1.0060x; 1.0060x over previous
"""CirculantLinear kernel for Trainium2 (Bass/Tile), 8-core data-parallel.

Math: reference computes out = IFFT_ortho( FFT(eigens) * FFT_ortho(x) ) summed
over input blocks.  Because the ortho norms cancel against the unnormalized
eigen-FFT, this is exactly a block-circulant real matmul applied to both the
real and imaginary channel of x:

    out[b, y*8+i2, c] = sum_{gx,m} W[y*8+i2, gx*8+m] * x[b, gx*8+m, c]
    W[o, i] = eigens[o//8, i//8, (o-i) % 8]

so the whole op is a dense 1024x1024 matmul on the TensorEngine (fp16 operands,
fp32 PSUM accumulation), sharded over the batch dim across 8 NeuronCores.

The circulant expansion of eigens into W happens ON DEVICE: the host only
passes a doubled+transposed copy of eigens (layout transform), and 8 strided
VectorE copies build the fp16 weight tiles (the mod-8 wrap becomes a plain
sliding window on the doubled axis).
"""

import os
import sys

import numpy as np

# Runtime deps (available via PYTHONPATH in the container; add fallbacks so the
# kernel is importable from a bare directory too).
for _p in ("/root/.axon_site/_ro/trn_rl_repo", "/root/.axon_site/_ro/pypackages",
           "/opt/trn_rl_repo", "/opt/pypackages"):
    if os.path.isdir(_p) and _p not in sys.path:
        sys.path.append(_p)

import concourse.bacc as bacc
import concourse.mybir as mybir
import concourse.tile as tile
from concourse import bass_utils

# Problem constants (hardcoded per contract).
B, IN, OUT, K = 8192, 1024, 1024, 8
GX, GY = IN // K, OUT // K  # 128, 128
N_CORES = 8
B_CORE = B // N_CORES  # 1024
P = 128  # partitions

F32 = mybir.dt.float32
F16 = mybir.dt.float16


def build_nc(b_core: int = B_CORE):
    """Build the per-core Bass program (SPMD: same program, different x shard)."""
    assert b_core % P == 0
    n_bt = b_core // P

    nc = bacc.Bacc("TRN2", target_bir_lowering=False, debug=False)

    x_dram = nc.dram_tensor("x", [b_core, 2 * IN], F32, kind="ExternalInput")
    # e2t[gx, y*16 + d] = eigens[y, gx, d % 8]   (doubled k-axis, gx-major)
    e2t_dram = nc.dram_tensor("e2t", [GX, GY * 2 * K], F16, kind="ExternalInput")
    # fp16 output: PSUM evac casts fp32->fp16; host widens back to fp32.
    # Halves store traffic (the DMA is the binding resource at the ridge).
    out_dram = nc.dram_tensor("out", [b_core, 2 * OUT], F16, kind="ExternalOutput")

    with tile.TileContext(nc) as tc:
        with (
            tc.tile_pool(name="const", bufs=1) as constp,
            tc.tile_pool(name="xraw", bufs=2) as xrawp,
            tc.tile_pool(name="xm", bufs=2) as xmp,
            tc.tile_pool(name="xt", bufs=2) as xtp,
            tc.tile_pool(name="outp", bufs=8) as outp,
            tc.tile_pool(name="psum", bufs=8, space="PSUM") as psump,
        ):
            # ALL DMAs ride the single SP (HWDGE) queue in an explicit static
            # order; the sequencer's head-of-line blocking turns that order
            # into a static schedule: e16, x0, T0, x1, T1, x2, x3, T2, ...
            # with every store deferred to the end (stores have no deadline).
            groups = [1, 2, 2, 2, 1] if n_bt == 8 else [1] * n_bt
            g_base = []
            b0 = 0
            for g in groups:
                g_base.append(b0)
                b0 += g

            e2t = constp.tile([P, GY * 2 * K], F16, name="e2t_sb")
            e2t_v = e2t.rearrange("g (y d) -> g y d", d=2 * K)  # [128,128,16]


            # first group's load up front (before e2t: x0 feeds the critical
            # cast->transpose chain); later loads are emitted after the
            # previous group's transpose (FIFO => auto-throttled prefetch)
            xraws = [None] * n_bt

            def load(bt):
                xr = xrawp.tile([P, 2 * IN], F32, name="xraw")
                nc.sync.dma_start(xr, x_dram.ap()[bt * P : (bt + 1) * P, :])
                xraws[bt] = xr

            for bt in range(g_base[1] if len(groups) > 1 else n_bt):
                load(bt)
            nc.scalar.dma_start(e2t, e2t_dram.ap())

            # PE warm-up: the DMA prologue leaves the PE idle for >10us which
            # would leave the HAM clock-gate cold for the first real matmul
            # group; burn dummy matmuls on a zeroed scratch meanwhile.
            wscr = constp.tile([P, 640], F16, name="wscr")
            nc.scalar.memzero(wscr[:])
            wps = psump.tile([P, 512], F32, name="wps", tag="ps")
            import os as _os2
            n_warm = int(_os2.environ.get("CIRC_WARM", "16"))
            for _ in range(n_warm):
                nc.tensor.matmul(wps, wscr[:, 0:P],
                                 wscr[:, P : P + 512], start=True, stop=True)

            # weights wt_m[gx, y*8+i2] = e[y, gx, (i2-m)%8]: one tile per m so
            # each matmul group only depends on the chunks it reads; (i2-m)%8
            # becomes a plain sliding window on the doubled axis.  All eight
            # copies run on the DVE, emitted after the first group's casts.
            wt_mo = []
            for m in range(K):
                wtm = constp.tile([P, OUT], F16, name="wtm", tag=f"wtm{m}")
                wt_mo.append(wtm)

            def emit_wt(ms):
                for m in ms:
                    src = e2t_v[:, :, K - m : 2 * K - m]
                    dst = wt_mo[m].rearrange("g (y i) -> g y i", i=K)
                    nc.vector.tensor_copy(dst, src)

            stores = []  # deferred (out_dram_slice, out_sb_slice, engine)

            for gidx, g_sz in enumerate(groups):
                base = g_base[gidx]
                # casts: deinterleave re/im + fp32->fp16 + reorder
                #   in  free idx: gx*16 + m*2 + c
                #   out free idx: ((c*8+m)*G + gi)*128 + gx
                nbufs = min(2, groups.count(g_sz))
                xm = xmp.tile([P, g_sz * 2 * IN], F16, name="xm", tag=f"xm{g_sz}",
                              bufs=nbufs)
                xm_v = xm.rearrange("b (c m i g) -> b c m i g", c=2, m=K, i=g_sz)
                for gi in range(g_sz):
                    xraw_v = xraws[base + gi].rearrange(
                        "b (g m c) -> b c m g", g=GX, m=K, c=2
                    )
                    for c in range(2):
                        nc.vector.tensor_copy(xm_v[:, c, :, gi], xraw_v[:, c])
                if gidx == 0:
                    emit_wt(range(K))

                # xbar transpose(s): [b, f] -> [gx, f//128, b]; first group per
                # channel so c0's matmuls start asap
                xt = xtp.tile([P, g_sz * 2 * K, P], F16, name="xt", tag=f"xt{g_sz}",
                              bufs=nbufs)
                with tc.high_priority():
                    if gidx == 0:
                        half = g_sz * K * P
                        nc.sync.dma_start(xt[:, 0 : g_sz * K, :], xm[:, 0:half],
                                          transpose=True)
                        nc.sync.dma_start(xt[:, g_sz * K : 2 * g_sz * K, :],
                                          xm[:, half : 2 * half], transpose=True)
                    else:
                        nc.sync.dma_start(xt, xm, transpose=True)
                xt_v = xt.rearrange("g (c m i) b -> g c m i b", c=2, m=K)

                if gidx + 1 < len(groups):
                    nxt = g_base[gidx + 1]
                    for bt in range(nxt, nxt + groups[gidx + 1]):
                        load(bt)


                for gi in range(g_sz):
                    bt = base + gi
                    out_sb = outp.tile([P, 2 * OUT], F16, name="out_sb")
                    out_v = out_sb.rearrange("b (o c) -> b c o", c=2)
                    last = bt == n_bt - 1
                    for c in range(2):
                        ps0 = psump.tile([P, 512], F32, name="ps0", tag="ps")
                        ps1 = psump.tile([P, 512], F32, name="ps1", tag="ps")
                        if last and c == 1:
                            # de-interleave the final groups: ps0's evac+store
                            # overlaps ps1's matmuls (shorter kernel tail)
                            for m in range(K):
                                nc.tensor.matmul(ps0, xt_v[:, c, m, gi],
                                                 wt_mo[m][:, 0:512],
                                                 start=(m == 0), stop=(m == K - 1))
                            nc.scalar.copy(out_v[:, c, 0:512], ps0)
                            for m in range(K):
                                nc.tensor.matmul(ps1, xt_v[:, c, m, gi],
                                                 wt_mo[m][:, 512:1024],
                                                 start=(m == 0), stop=(m == K - 1))
                            nc.vector.tensor_copy(out_v[:, c, 512:1024], ps1)
                            continue
                        for m in range(K):
                            lhsT = xt_v[:, c, m, gi]  # [gx=128, b=128]
                            nc.tensor.matmul(
                                ps0, lhsT, wt_mo[m][:, 0:512],
                                start=(m == 0), stop=(m == K - 1),
                            )
                            nc.tensor.matmul(
                                ps1, lhsT, wt_mo[m][:, 512:1024],
                                start=(m == 0), stop=(m == K - 1),
                            )
                        # PSUM -> SBUF re-interleaving channels (strided write)
                        nc.scalar.copy(out_v[:, c, 0:512], ps0)
                        nc.scalar.copy(out_v[:, c, 512:1024], ps1)

                    n_pieces = 4 if last else 2
                    step = (2 * OUT) // n_pieces
                    for pc in range(n_pieces):
                        stores.append((
                            (bt * P, pc * step, (pc + 1) * step),
                            out_sb[:, pc * step : (pc + 1) * step],
                        ))

            # deferred stores, in order, at the back of the SP queue
            for (row0, c0_, c1_), src in stores:
                nc.sync.dma_start(out_dram.ap()[row0 : row0 + P, c0_:c1_], src)

    nc.compile()
    return nc


_NC_CACHE = {}


def _get_nc(b_core: int = B_CORE):
    if b_core not in _NC_CACHE:
        _NC_CACHE[b_core] = build_nc(b_core)
    return _NC_CACHE[b_core]


def make_e2t(eigens: np.ndarray) -> np.ndarray:
    """Host layout transform: doubled k-axis, gx-major.  No arithmetic."""
    e2 = np.concatenate([eigens, eigens], axis=-1)  # [GY, GX, 16]
    return np.ascontiguousarray(e2.transpose(1, 0, 2)).reshape(GX, GY * 2 * K).astype(np.float16)


def kernel(x: np.ndarray, eigens: np.ndarray, _trace: bool = False):
    assert x.shape == (B, IN, 2) and eigens.shape == (GY, GX, K)
    nc = _get_nc()
    e2t = make_e2t(np.asarray(eigens, dtype=np.float32))
    x = np.asarray(x, dtype=np.float32)
    in_maps = []
    for ci in range(N_CORES):
        xs = np.ascontiguousarray(x[ci * B_CORE : (ci + 1) * B_CORE]).reshape(B_CORE, 2 * IN)
        in_maps.append({"x": xs, "e2t": e2t})
    res = bass_utils.run_bass_kernel_spmd(
        nc, in_maps, core_ids=list(range(N_CORES)), trace=_trace
    )
    out = np.concatenate(
        [r["out"].astype(np.float32).reshape(B_CORE, OUT, 2) for r in res.results],
        axis=0,
    )
    if _trace:
        return out, res
    return out



# revision 47
# speedup vs baseline: 1.0163x; 1.0103x over previous
"""CirculantLinear kernel for Trainium2 (Bass/Tile), 8-core data-parallel.

Math: reference computes out = IFFT_ortho( FFT(eigens) * FFT_ortho(x) ) summed
over input blocks.  Because the ortho norms cancel against the unnormalized
eigen-FFT, this is exactly a block-circulant real matmul applied to both the
real and imaginary channel of x:

    out[b, y*8+i2, c] = sum_{gx,m} W[y*8+i2, gx*8+m] * x[b, gx*8+m, c]
    W[o, i] = eigens[o//8, i//8, (o-i) % 8]

so the whole op is a dense 1024x1024 matmul on the TensorEngine (fp16 operands,
fp32 PSUM accumulation), sharded over the batch dim across 8 NeuronCores.

The circulant expansion of eigens into W happens ON DEVICE: the host only
passes a doubled+transposed copy of eigens (layout transform), and 8 strided
VectorE copies build the fp16 weight tiles (the mod-8 wrap becomes a plain
sliding window on the doubled axis).
"""

import os
import sys

import numpy as np

# Runtime deps (available via PYTHONPATH in the container; add fallbacks so the
# kernel is importable from a bare directory too).
for _p in ("/root/.axon_site/_ro/trn_rl_repo", "/root/.axon_site/_ro/pypackages",
           "/opt/trn_rl_repo", "/opt/pypackages"):
    if os.path.isdir(_p) and _p not in sys.path:
        sys.path.append(_p)

import concourse.bacc as bacc
import concourse.mybir as mybir
import concourse.tile as tile
from concourse import bass_utils

# Problem constants (hardcoded per contract).
B, IN, OUT, K = 8192, 1024, 1024, 8
GX, GY = IN // K, OUT // K  # 128, 128
N_CORES = 8
B_CORE = B // N_CORES  # 1024
P = 128  # partitions

F32 = mybir.dt.float32
F16 = mybir.dt.float16


def build_nc(b_core: int = B_CORE):
    """Build the per-core Bass program (SPMD: same program, different x shard)."""
    assert b_core % P == 0
    n_bt = b_core // P

    nc = bacc.Bacc("TRN2", target_bir_lowering=False, debug=False)

    x_dram = nc.dram_tensor("x", [b_core, 2 * IN], F32, kind="ExternalInput")
    # e2t[gx, y*16 + d] = eigens[y, gx, d % 8]   (doubled k-axis, gx-major)
    e2t_dram = nc.dram_tensor("e2t", [GX, GY * 2 * K], F16, kind="ExternalInput")
    # fp16 output: PSUM evac casts fp32->fp16; host widens back to fp32.
    # Halves store traffic (the DMA is the binding resource at the ridge).
    out_dram = nc.dram_tensor("out", [b_core, 2 * OUT], F16, kind="ExternalOutput")

    with tile.TileContext(nc) as tc:
        with (
            tc.tile_pool(name="const", bufs=1) as constp,
            tc.tile_pool(name="xraw", bufs=2) as xrawp,
            tc.tile_pool(name="xm", bufs=2) as xmp,
            tc.tile_pool(name="xt", bufs=2) as xtp,
            tc.tile_pool(name="outp", bufs=8) as outp,
            tc.tile_pool(name="psum", bufs=8, space="PSUM") as psump,
        ):
            # ALL DMAs ride the single SP (HWDGE) queue in an explicit static
            # order; the sequencer's head-of-line blocking turns that order
            # into a static schedule: e16, x0, T0, x1, T1, x2, x3, T2, ...
            # with every store deferred to the end (stores have no deadline).
            groups = [1, 2, 2, 2, 1] if n_bt == 8 else [1] * n_bt
            g_base = []
            b0 = 0
            for g in groups:
                g_base.append(b0)
                b0 += g

            e2t = constp.tile([P, GY * 2 * K], F16, name="e2t_sb")
            e2t_v = e2t.rearrange("g (y d) -> g y d", d=2 * K)  # [128,128,16]


            # first group's load up front (before e2t: x0 feeds the critical
            # cast->transpose chain); later loads are emitted after the
            # previous group's transpose (FIFO => auto-throttled prefetch)
            xraws = [None] * n_bt

            def load(bt):
                xr = xrawp.tile([P, 2 * IN], F32, name="xraw")
                nc.sync.dma_start(xr, x_dram.ap()[bt * P : (bt + 1) * P, :])
                xraws[bt] = xr

            for bt in range(g_base[1] if len(groups) > 1 else n_bt):
                load(bt)
            nc.scalar.dma_start(e2t, e2t_dram.ap())

            # PE warm-up: the DMA prologue leaves the PE idle for >10us which
            # would leave the HAM clock-gate cold for the first real matmul
            # group; burn dummy matmuls on a zeroed scratch meanwhile.
            wscr = constp.tile([P, 640], F16, name="wscr")
            nc.scalar.memzero(wscr[:])
            wps = psump.tile([P, 512], F32, name="wps", tag="ps")
            for _ in range(16):
                nc.tensor.matmul(wps, wscr[:, 0:P],
                                 wscr[:, P : P + 512], start=True, stop=True)

            # weights wt_m[gx, y*8+i2] = e[y, gx, (i2-m)%8]: one tile per m so
            # each matmul group only depends on the chunks it reads; (i2-m)%8
            # becomes a plain sliding window on the doubled axis.  All eight
            # copies run on the DVE, emitted after the first group's casts.
            wt_mo = []
            for m in range(K):
                wtm = constp.tile([P, OUT], F16, name="wtm", tag=f"wtm{m}")
                wt_mo.append(wtm)

            def emit_wt(ms):
                for m in ms:
                    src = e2t_v[:, :, K - m : 2 * K - m]
                    dst = wt_mo[m].rearrange("g (y i) -> g y i", i=K)
                    nc.vector.tensor_copy(dst, src)

            stores = []  # deferred (out_dram_slice, out_sb_slice, engine)

            for gidx, g_sz in enumerate(groups):
                base = g_base[gidx]
                # casts: deinterleave re/im + fp32->fp16 + reorder
                #   in  free idx: gx*16 + m*2 + c
                #   out free idx: ((c*8+m)*G + gi)*128 + gx
                nbufs = min(2, groups.count(g_sz))
                xm = xmp.tile([P, g_sz * 2 * IN], F16, name="xm", tag=f"xm{g_sz}",
                              bufs=nbufs)
                xm_v = xm.rearrange("b (c m i g) -> b c m i g", c=2, m=K, i=g_sz)
                for gi in range(g_sz):
                    xraw_v = xraws[base + gi].rearrange(
                        "b (g m c) -> b c m g", g=GX, m=K, c=2
                    )
                    for c in range(2):
                        nc.vector.tensor_copy(xm_v[:, c, :, gi], xraw_v[:, c])
                if gidx == 0:
                    emit_wt(range(K))

                # xbar transpose(s): [b, f] -> [gx, f//128, b]; first group per
                # channel so c0's matmuls start asap
                xt = xtp.tile([P, g_sz * 2 * K, P], F16, name="xt", tag=f"xt{g_sz}",
                              bufs=nbufs)
                with tc.high_priority():
                    if gidx == 0:
                        half = g_sz * K * P
                        nc.sync.dma_start(xt[:, 0 : g_sz * K, :], xm[:, 0:half],
                                          transpose=True)
                        nc.sync.dma_start(xt[:, g_sz * K : 2 * g_sz * K, :],
                                          xm[:, half : 2 * half], transpose=True)
                    else:
                        nc.sync.dma_start(xt, xm, transpose=True)
                xt_v = xt.rearrange("g (c m i) b -> g c m i b", c=2, m=K)

                if gidx + 1 < len(groups):
                    nxt = g_base[gidx + 1]
                    for bt in range(nxt, nxt + groups[gidx + 1]):
                        load(bt)


                for gi in range(g_sz):
                    bt = base + gi
                    out_sb = outp.tile([P, 2 * OUT], F16, name="out_sb")
                    out_v = out_sb.rearrange("b (o c) -> b c o", c=2)
                    last = bt == n_bt - 1
                    for c in range(2):
                        if bt == 0:
                            # quarter-width matmuls while the PE p-state is
                            # still cold: the ramp phase costs half as much
                            for q in range(4):
                                psq = psump.tile([P, 256], F32, name="psq",
                                                 tag="ps")
                                for m in range(K):
                                    nc.tensor.matmul(
                                        psq, xt_v[:, c, m, gi],
                                        wt_mo[m][:, q * 256 : (q + 1) * 256],
                                        start=(m == 0), stop=(m == K - 1),
                                    )
                                nc.scalar.copy(
                                    out_v[:, c, q * 256 : (q + 1) * 256], psq
                                )
                            continue
                        ps0 = psump.tile([P, 512], F32, name="ps0", tag="ps")
                        ps1 = psump.tile([P, 512], F32, name="ps1", tag="ps")
                        if last and c == 1:
                            # de-interleave the final groups: ps0's evac+store
                            # overlaps ps1's matmuls (shorter kernel tail)
                            for m in range(K):
                                nc.tensor.matmul(ps0, xt_v[:, c, m, gi],
                                                 wt_mo[m][:, 0:512],
                                                 start=(m == 0), stop=(m == K - 1))
                            nc.scalar.copy(out_v[:, c, 0:512], ps0)
                            for m in range(K):
                                nc.tensor.matmul(ps1, xt_v[:, c, m, gi],
                                                 wt_mo[m][:, 512:1024],
                                                 start=(m == 0), stop=(m == K - 1))
                            nc.vector.tensor_copy(out_v[:, c, 512:1024], ps1)
                            continue
                        for m in range(K):
                            lhsT = xt_v[:, c, m, gi]  # [gx=128, b=128]
                            nc.tensor.matmul(
                                ps0, lhsT, wt_mo[m][:, 0:512],
                                start=(m == 0), stop=(m == K - 1),
                            )
                            nc.tensor.matmul(
                                ps1, lhsT, wt_mo[m][:, 512:1024],
                                start=(m == 0), stop=(m == K - 1),
                            )
                        # PSUM -> SBUF re-interleaving channels (strided write)
                        nc.scalar.copy(out_v[:, c, 0:512], ps0)
                        nc.scalar.copy(out_v[:, c, 512:1024], ps1)

                    n_pieces = 4 if last else 2
                    step = (2 * OUT) // n_pieces
                    for pc in range(n_pieces):
                        stores.append((
                            (bt * P, pc * step, (pc + 1) * step),
                            out_sb[:, pc * step : (pc + 1) * step],
                        ))

            # deferred stores, in order, at the back of the SP queue
            for (row0, c0_, c1_), src in stores:
                nc.sync.dma_start(out_dram.ap()[row0 : row0 + P, c0_:c1_], src)

    nc.compile()
    return nc


_NC_CACHE = {}


def _get_nc(b_core: int = B_CORE):
    if b_core not in _NC_CACHE:
        _NC_CACHE[b_core] = build_nc(b_core)
    return _NC_CACHE[b_core]


def make_e2t(eigens: np.ndarray) -> np.ndarray:
    """Host layout transform: doubled k-axis, gx-major.  No arithmetic."""
    e2 = np.concatenate([eigens, eigens], axis=-1)  # [GY, GX, 16]
    return np.ascontiguousarray(e2.transpose(1, 0, 2)).reshape(GX, GY * 2 * K).astype(np.float16)


def kernel(x: np.ndarray, eigens: np.ndarray, _trace: bool = False):
    assert x.shape == (B, IN, 2) and eigens.shape == (GY, GX, K)
    nc = _get_nc()
    e2t = make_e2t(np.asarray(eigens, dtype=np.float32))
    x = np.asarray(x, dtype=np.float32)
    in_maps = []
    for ci in range(N_CORES):
        xs = np.ascontiguousarray(x[ci * B_CORE : (ci + 1) * B_CORE]).reshape(B_CORE, 2 * IN)
        in_maps.append({"x": xs, "e2t": e2t})
    res = bass_utils.run_bass_kernel_spmd(
        nc, in_maps, core_ids=list(range(N_CORES)), trace=_trace
    )
    out = np.concatenate(
        [r["out"].astype(np.float32).reshape(B_CORE, OUT, 2) for r in res.results],
        axis=0,
    )
    if _trace:
        return out, res
    return out

